# revision 15
# baseline (speedup 1.0000x reference)
"""Multi-head attention (B=2, S=2048, E=1024, H=16) on 8 Trainium2 NeuronCores.

Sharding: tensor-parallel over heads — core i owns heads (2i, 2i+1).
  Phase A  (per core, per batch): q/k/v projections for its 2 heads,
            feature-major [128 = 2x64 head-features, tokens]; v is
            PE-transposed to token-major with a ones column appended per
            head (softmax-denominator trick) and zero padding to 128
            stationary columns per head (enables fast weight loads).
  Phase B/C (per core, per batch): scores^T for both heads via K=64 PE row
            tiles at row positions 0/64 (no zero-padding waste); exp on
            ScalarE straight out of PSUM (softmax without max-subtraction —
            scores are O(1) for these inputs); AV matmul with the ones
            column so the denominator falls out of the same fp32
            accumulation; normalize = one partition-64->0 DMA hop, gpsimd
            partition-broadcast of the raw denominators, full-lane DVE
            approx-reciprocal, fused multiply into the AllToAll staging
            buffer.
  AllToAll: bf16 collectives re-sharding head-parallel [128 feat, tokens]
            to token-parallel [all 1024 feat, tokens/8]. Tokens map to
            cores in 128-token granules, block = granule % 8, so batch 1's
            collective can split into two slot-halves: the first half
            overlaps the last attention q-tiles, the second overlaps the
            (deferred) batch-0 output projection.
  Phase D  (per core, per batch, per slot): output projection for one
            128-token granule; emitted last so it backfills PE idle time
            during the final collective.

Batches are emitted interleaved so the Tile scheduler fills ScalarE-bound
gaps in one batch's attention with the other batch's projection matmuls.
Matmuls run in bf16 (full-rate + FWL weight loads); inputs are cast to bf16
on the host; PSUM accumulation is fp32 throughout.
"""

import numpy as np
import ml_dtypes

import concourse.bass as bass
import concourse.mybir as mybir
import concourse.tile as tile
from concourse import bacc
from concourse import bass_utils
from concourse.masks import make_identity

F32 = mybir.dt.float32
BF16 = mybir.dt.bfloat16
F32R = mybir.dt.float32r
N_CORES = 8
P = 128

COMPUTE = "bf16"              # "bf16" (fast) or "f32r" (precise fallback)

# Full problem dims (hardcoded per the harness contract)
B_FULL, S_FULL, E, H, D = 2, 2048, 1024, 16, 64
HPC = H // N_CORES            # heads per core = 2
F = HPC * D                   # feature cols per core = 128
SCALE = D ** -0.5
GR = 128                      # token granule for output sharding


def build_nc(B=B_FULL, S=S_FULL, compute=COMPUTE):
    CDT = BF16 if compute == "bf16" else F32R
    IN_DT = BF16 if compute == "bf16" else F32
    T = B * S                 # tokens
    KO = E // P               # 8 contraction chunks over embed
    TC = min(512, S)          # phase-A token chunk
    NTC = S // TC             # chunks per batch
    Q2 = min(512, S)          # q tile
    NQ = S // Q2
    KC = S // P               # k chunks per batch
    G4 = max(1, 1024 // Q2)   # kc group per exp call (N=1024 per ACTIVATE)
    TPB = S // N_CORES        # tokens per core PER BATCH for output proj
    SLOTS = TPB // GR         # granule slots per core per batch (2)

    nc = bacc.Bacc("TRN2", target_bir_lowering=False, debug=False,
                   num_devices=N_CORES)

    xT = nc.dram_tensor("xT", [E, T], IN_DT, kind="ExternalInput").ap()
    wq = nc.dram_tensor("wq", [E, F], IN_DT, kind="ExternalInput").ap()
    wk = nc.dram_tensor("wk", [E, F], IN_DT, kind="ExternalInput").ap()
    wv = nc.dram_tensor("wv", [E, F], IN_DT, kind="ExternalInput").ap()
    bq = nc.dram_tensor("bq", [F, 1], F32, kind="ExternalInput").ap()
    bk = nc.dram_tensor("bk", [F, 1], F32, kind="ExternalInput").ap()
    bv = nc.dram_tensor("bv", [F, 1], F32, kind="ExternalInput").ap()
    ow = nc.dram_tensor("ow", [E, E], IN_DT, kind="ExternalInput").ap()
    ob = nc.dram_tensor("ob", [1, E], F32, kind="ExternalInput").ap()
    # rows = [b, slot s] -> batch-b tokens [(core + 8*s)*GR, +GR)
    out = nc.dram_tensor("out", [B * TPB, E], F32, kind="ExternalOutput").ap()

    Exp = mybir.ActivationFunctionType.Exp

    with tile.TileContext(nc) as tc:
        with tc.tile_pool(name="persist", bufs=1) as persist, \
             tc.tile_pool(name="pAw", bufs=1) as pAw, \
             tc.tile_pool(name="pA", bufs=3) as pA, \
             tc.tile_pool(name="pBC", bufs=2) as pBC, \
             tc.tile_pool(name="pNr", bufs=2) as pNr, \
             tc.tile_pool(name="pD", bufs=1) as pD, \
             tc.tile_pool(name="pDo", bufs=2) as pDo, \
             tc.tile_pool(name="psA", bufs=1, space="PSUM") as psA, \
             tc.tile_pool(name="psT", bufs=1, space="PSUM") as psT, \
             tc.tile_pool(name="psS", bufs=2, space="PSUM") as psS, \
             tc.tile_pool(name="psAV", bufs=2, space="PSUM") as psAV, \
             tc.tile_pool(name="dramp", bufs=1, space="DRAM") as dramp:
            ident = persist.tile([P, P], CDT)
            make_identity(nc, ident)
            bq_sb = persist.tile([P, 1], F32)
            bk_sb = persist.tile([P, 1], F32)
            bv_sb = persist.tile([P, 1], F32)
            nc.sync.dma_start(bq_sb, bq)
            nc.sync.dma_start(bk_sb, bk)
            nc.sync.dma_start(bv_sb, bv)
            ob_row = persist.tile([1, E], F32)
            nc.sync.dma_start(ob_row, ob)
            obb = persist.tile([P, E], F32)
            nc.gpsimd.partition_broadcast(obb, ob_row)

            qfm = persist.tile([P, T], CDT)     # q^T (both heads stacked)
            kfm = persist.tile([P, T], CDT)     # k^T (rows 0:64 A, 64:128 B)
            # v token-major per 128-token chunk; per head 128 stationary
            # cols: [v (64) | ones | tail], head A tail = head B v rows,
            # head B tail = zeros. Head A lhsT = cols 0:128, B = 65:193.
            vtm = persist.tile([P, T // P, 194], CDT)
            ones1 = persist.tile([P, 1], F32)
            nc.vector.memset(ones1, 1.0)
            nc.vector.tensor_copy(vtm[:, :, 64], ones1.to_broadcast([P, T // P]))
            nc.vector.tensor_copy(vtm[:, :, 129], ones1.to_broadcast([P, T // P]))
            nc.vector.memset(vtm[:, :, 130:194], 0.0)
            attnA = persist.tile([64, T], CDT)  # head A attn out^T (normalized)
            attnB = persist.tile([64, T], CDT)

            wq_sb = pAw.tile([P, KO, F], CDT)
            wk_sb = pAw.tile([P, KO, F], CDT)
            wv_sb = pAw.tile([P, KO, F], CDT)
            nc.sync.dma_start(wq_sb, wq.rearrange("(ko p) f -> p ko f", p=P))
            xTr = xT.rearrange("(ko p) t -> p ko t", p=P)
            wkv_loaded = []

            # all-to-all staging: block r, slot s -> batch tokens of
            # granule g = r + 8*s; one contiguous buffer per (batch, slot)
            a2a_in = [[dramp.tile([N_CORES, P, GR], CDT, name=f"a2a_in{b}{s}")
                       for s in range(SLOTS)] for b in range(B)]
            a2a_out = [[dramp.tile([N_CORES, P, GR], CDT,
                                   name=f"a2a_out{b}{s}")
                        for s in range(SLOTS)] for b in range(B)]

            def phase_a(b):
                for tcx in range(NTC):
                    t0 = b * S + tcx * TC
                    xt = pA.tile([P, KO, TC], CDT, tag="xt")
                    if b == 0 and tcx == 0:
                        # split the first chunk's load per-ko so the very
                        # first matmul starts after ~128KB instead of ~1MB
                        for ko in range(KO):
                            nc.sync.dma_start(xt[:, ko],
                                              xTr[:, ko, t0:t0 + TC])
                    else:
                        nc.sync.dma_start(xt, xTr[:, :, t0:t0 + TC])
                    if not wkv_loaded:
                        nc.sync.dma_start(
                            wk_sb, wk.rearrange("(ko p) f -> p ko f", p=P))
                        nc.sync.dma_start(
                            wv_sb, wv.rearrange("(ko p) f -> p ko f", p=P))
                        wkv_loaded.append(True)
                    ps = psA.tile([P, TC], F32, tag="ps")
                    for ko in range(KO):
                        nc.tensor.matmul(ps, lhsT=wq_sb[:, ko], rhs=xt[:, ko],
                                         start=(ko == 0), stop=(ko == KO - 1))
                    nc.vector.tensor_scalar_add(qfm[:, t0:t0 + TC], ps, bq_sb)
                    ps = psT.tile([P, TC], F32, tag="tr")
                    for ko in range(KO):
                        nc.tensor.matmul(ps, lhsT=wk_sb[:, ko], rhs=xt[:, ko],
                                         start=(ko == 0), stop=(ko == KO - 1))
                    nc.vector.tensor_scalar_add(kfm[:, t0:t0 + TC], ps, bk_sb)
                    ps = psA.tile([P, TC], F32, tag="ps")
                    for ko in range(KO):
                        nc.tensor.matmul(ps, lhsT=wv_sb[:, ko], rhs=xt[:, ko],
                                         start=(ko == 0), stop=(ko == KO - 1))
                    vfm = pA.tile([P, TC], CDT, tag="vfm")
                    nc.vector.tensor_scalar_add(vfm, ps, bv_sb)
                    for sub in range(TC // P):
                        pst = psT.tile([P, P], CDT, tag="tr")
                        nc.tensor.transpose(pst, vfm[:, sub * P:(sub + 1) * P],
                                            ident)
                        c = (t0 + sub * P) // P
                        nc.vector.tensor_copy(vtm[:, c, 0:64], pst[:, 0:64])
                        nc.vector.tensor_copy(vtm[:, c, 65:129], pst[:, 64:128])

            def phase_bc(b, q_lo, q_hi):
                for qi in range(q_lo, q_hi):
                    q0 = b * S + qi * Q2
                    eA = pBC.tile([P, KC, Q2], CDT, tag="expA")
                    eB = pBC.tile([P, KC, Q2], CDT, tag="expB")
                    for kg in range(KC // G4):
                        sA = psS.tile([P, G4, Q2], F32, tag="sS")
                        sB = psS.tile([P, G4, Q2], F32, tag="sS")
                        for j in range(G4):
                            kc = kg * G4 + j
                            k0 = b * S + kc * P
                            # both heads: K=64 row tiles at partitions 0/64
                            nc.tensor.matmul(
                                sA[:, j], lhsT=kfm[0:64, k0:k0 + P],
                                rhs=qfm[0:64, q0:q0 + Q2],
                                start=True, stop=True)
                            nc.tensor.matmul(
                                sB[:, j], lhsT=kfm[64:128, k0:k0 + P],
                                rhs=qfm[64:128, q0:q0 + Q2],
                                start=True, stop=True)
                        g0 = kg * G4
                        nc.scalar.activation(eA[:, g0:g0 + G4], sA, Exp,
                                             scale=SCALE)
                        nc.scalar.activation(eB[:, g0:g0 + G4], sB, Exp,
                                             scale=SCALE)
                    pvA = psAV.tile([P, Q2], F32, tag="av")
                    pvB = psAV.tile([P, Q2], F32, tag="av")
                    for kc in range(KC):
                        c = (b * S) // P + kc
                        nc.tensor.matmul(pvA, lhsT=vtm[:, c, 0:128],
                                         rhs=eA[:, kc],
                                         start=(kc == 0), stop=(kc == KC - 1))
                        nc.tensor.matmul(pvB, lhsT=vtm[:, c, 65:193],
                                         rhs=eB[:, kc],
                                         start=(kc == 0), stop=(kc == KC - 1))
                    # row 64 = softmax denominators (raw); 65:128 garbage/0
                    dsb = pNr.tile([P, 2, Q2], F32, tag="dsb")
                    nc.vector.tensor_copy(dsb[64:65, 0], pvA[64:65])
                    nc.vector.tensor_copy(dsb[64:65, 1], pvB[64:65])
                    den0 = pNr.tile([1, 2, Q2], F32, tag="den0")
                    nc.sync.dma_start(den0, dsb[64:65])   # partition 64 -> 0
                    den = pNr.tile([64, 2, Q2], F32, tag="den")
                    nc.gpsimd.partition_broadcast(den, den0)
                    nc.vector.reciprocal_approx_fast(den, den)
                    nc.vector.tensor_mul(attnA[:, q0:q0 + Q2], pvA[0:64],
                                         den[:, 0])
                    nc.vector.tensor_mul(attnB[:, q0:q0 + Q2], pvB[0:64],
                                         den[:, 1])
                    for j in range(Q2 // GR):
                        g = qi * (Q2 // GR) + j    # batch-local granule
                        r, sl = g % N_CORES, g // N_CORES
                        gq = b * S + g * GR
                        nc.sync.dma_start(a2a_in[b][sl][r, 0:64],
                                          attnA[:, gq:gq + GR])
                        nc.sync.dma_start(a2a_in[b][sl][r, 64:128],
                                          attnB[:, gq:gq + GR])

            def send_a2a(b, sl):
                nc.gpsimd.collective_compute(
                    "AllToAll", mybir.AluOpType.bypass,
                    replica_groups=[list(range(N_CORES))],
                    ins=[a2a_in[b][sl].opt()], outs=[a2a_out[b][sl].opt()])

            ow_sb = pD.tile([P, KO, E], CDT)

            def phase_d(b, sl):
                ga = pD.tile([P, N_CORES, GR], CDT, name=f"ga{b}{sl}")
                nc.sync.dma_start(ga, a2a_out[b][sl].rearrange("c p t -> p c t"))
                for n2 in range(E // 512):
                    if n2 % 2 == 0:
                        pso = psA.tile([P, 512], F32, tag="ps")
                    else:
                        pso = psT.tile([P, 512], F32, tag="tr")
                    for r in range(N_CORES):
                        nc.tensor.matmul(
                            pso, lhsT=ga[:, r],
                            rhs=ow_sb[:, r, n2 * 512:(n2 + 1) * 512],
                            start=(r == 0), stop=(r == N_CORES - 1))
                    osb = pDo.tile([GR, 512], F32, tag="osb")
                    nc.vector.tensor_add(osb, pso[0:GR],
                                         obb[0:GR, n2 * 512:(n2 + 1) * 512])
                    r0 = b * TPB + sl * GR
                    nc.sync.dma_start(
                        out[r0:r0 + GR, n2 * 512:(n2 + 1) * 512], osb)

            phase_a(0)
            phase_bc(0, 0, NQ)
            for sl in range(SLOTS):
                send_a2a(0, sl)
            nc.sync.dma_start(ow_sb, ow.rearrange("(r p) e -> p r e", p=P))
            if B > 1:
                phase_a(1)
                phase_bc(1, 0, NQ // 2)
                send_a2a(1, 0)
                phase_bc(1, NQ // 2, NQ)
                send_a2a(1, 1)
            for sl in range(SLOTS):
                phase_d(0, sl)
            if B > 1:
                for sl in range(SLOTS):
                    phase_d(1, sl)

    nc.compile()
    return nc


def make_in_maps(x, qkv_w, qkv_b, o_w, o_b, B=B_FULL, S=S_FULL,
                 compute=COMPUTE):
    """Host-side sharding: full inputs -> per-core input dicts."""
    T = B * S
    idt = ml_dtypes.bfloat16 if compute == "bf16" else np.float32
    x = np.asarray(x, dtype=np.float32)
    qkv_w = np.asarray(qkv_w, dtype=np.float32).astype(idt)
    qkv_b = np.asarray(qkv_b, dtype=np.float32)
    o_w = np.ascontiguousarray(np.asarray(o_w, dtype=np.float32).astype(idt))
    o_b = np.asarray(o_b, dtype=np.float32).reshape(1, E)
    xT = np.ascontiguousarray(x.reshape(T, E).T.astype(idt))
    in_maps = []
    for i in range(N_CORES):
        c0 = i * F
        in_maps.append({
            "xT": xT,
            "wq": np.ascontiguousarray(qkv_w[:, c0:c0 + F]),
            "wk": np.ascontiguousarray(qkv_w[:, E + c0:E + c0 + F]),
            "wv": np.ascontiguousarray(qkv_w[:, 2 * E + c0:2 * E + c0 + F]),
            "bq": np.ascontiguousarray(qkv_b[c0:c0 + F].reshape(F, 1)),
            "bk": np.ascontiguousarray(qkv_b[E + c0:E + c0 + F].reshape(F, 1)),
            "bv": np.ascontiguousarray(
                qkv_b[2 * E + c0:2 * E + c0 + F].reshape(F, 1)),
            "ow": o_w,
            "ob": o_b,
        })
    return in_maps


def gather_out(results, B=B_FULL, S=S_FULL):
    """Per-core [B*TPB, E] slices -> full [B, S, E].

    Core c, batch b, slot s rows map to batch tokens
    [(c + 8*s)*GR, (c + 8*s + 1)*GR).
    """
    TPB = S // N_CORES
    SLOTS = TPB // GR
    full = np.empty((B, S, E), dtype=np.float32)
    for c in range(N_CORES):
        r = results[c]["out"]
        for b in range(B):
            for s in range(SLOTS):
                g = c + N_CORES * s
                full[b, g * GR:(g + 1) * GR] = \
                    r[b * TPB + s * GR:b * TPB + (s + 1) * GR]
    return full


_NC_CACHE = {}


def _get_nc(B=B_FULL, S=S_FULL):
    key = (B, S, COMPUTE)
    if key not in _NC_CACHE:
        _NC_CACHE[key] = build_nc(B, S, COMPUTE)
    return _NC_CACHE[key]


def kernel(x, qkv_w, qkv_b, o_w, o_b):
    B, S, _ = np.asarray(x).shape
    nc = _get_nc(B, S)
    in_maps = make_in_maps(x, qkv_w, qkv_b, o_w, o_b, B, S)
    res = bass_utils.run_bass_kernel_spmd(
        nc, in_maps, core_ids=list(range(N_CORES)))
    return gather_out(res.results, B, S)


# revision 19
# speedup vs baseline: 1.2547x; 1.2547x over previous
"""Multi-head attention (B=2, S=2048, E=1024, H=16) on 8 Trainium2 NeuronCores.

Sharding: tensor-parallel over heads — core i owns heads (2i, 2i+1).
  Phase A  (per core, per batch): q/k/v projections for its 2 heads,
            feature-major [128 = 2x64 head-features, tokens]; v is
            PE-transposed to token-major with a ones column appended per
            head (softmax-denominator trick) and zero padding to 128
            stationary columns per head (enables fast weight loads).
  Phase B/C (per core, per batch): scores^T for both heads via K=64 PE row
            tiles at row positions 0/64 (no zero-padding waste); exp on
            ScalarE straight out of PSUM (softmax without max-subtraction —
            scores are O(1) for these inputs); AV matmul with the ones
            column so the denominator falls out of the same fp32
            accumulation; normalize = one partition-64->0 DMA hop, gpsimd
            partition-broadcast of the raw denominators, full-lane DVE
            approx-reciprocal, fused multiply into the AllToAll staging
            buffer.
  AllToAll: bf16 collectives re-sharding head-parallel [128 feat, tokens]
            to token-parallel [all 1024 feat, tokens/8]. Tokens map to
            cores in 128-token granules, block = granule % 8, so batch 1's
            collective can split into two slot-halves: the first half
            overlaps the last attention q-tiles, the second overlaps the
            (deferred) batch-0 output projection.
  Phase D  (per core, per batch, per slot): output projection for one
            128-token granule; emitted last so it backfills PE idle time
            during the final collective.

Batches are emitted interleaved so the Tile scheduler fills ScalarE-bound
gaps in one batch's attention with the other batch's projection matmuls.
Matmuls run in bf16 (full-rate + FWL weight loads); inputs are cast to bf16
on the host; PSUM accumulation is fp32 throughout.
"""

import numpy as np
import ml_dtypes

import concourse.bass as bass
import concourse.mybir as mybir
import concourse.tile as tile
from concourse import bacc
from concourse import bass_utils
from concourse.masks import make_identity

F32 = mybir.dt.float32
BF16 = mybir.dt.bfloat16
F32R = mybir.dt.float32r
N_CORES = 8
P = 128

COMPUTE = "bf16"              # "bf16" (fast) or "f32r" (precise fallback)

# Full problem dims (hardcoded per the harness contract)
B_FULL, S_FULL, E, H, D = 2, 2048, 1024, 16, 64
HPC = H // N_CORES            # heads per core = 2
F = HPC * D                   # feature cols per core = 128
SCALE = D ** -0.5
GR = 128                      # token granule for output sharding


def build_nc(B=B_FULL, S=S_FULL, compute=COMPUTE):
    CDT = BF16 if compute == "bf16" else F32R
    IN_DT = BF16 if compute == "bf16" else F32
    T = B * S                 # tokens
    KO = E // P               # 8 contraction chunks over embed
    TC = min(512, S)          # phase-A token chunk
    NTC = S // TC             # chunks per batch
    Q2 = min(512, S)          # q tile
    NQ = S // Q2
    KC = S // P               # k chunks per batch
    G4 = max(1, 1024 // Q2)   # kc group per exp call (N=1024 per ACTIVATE)
    TPB = S // N_CORES        # tokens per core PER BATCH for output proj
    SLOTS = TPB // GR         # granule slots per core per batch (2)

    nc = bacc.Bacc("TRN2", target_bir_lowering=False, debug=False,
                   num_devices=N_CORES)

    xT = nc.dram_tensor("xT", [E, T], IN_DT, kind="ExternalInput").ap()
    wq = nc.dram_tensor("wq", [E, F], IN_DT, kind="ExternalInput").ap()
    wk = nc.dram_tensor("wk", [E, F], IN_DT, kind="ExternalInput").ap()
    wv = nc.dram_tensor("wv", [E, F], IN_DT, kind="ExternalInput").ap()
    bq = nc.dram_tensor("bq", [F, 1], F32, kind="ExternalInput").ap()
    bk = nc.dram_tensor("bk", [F, 1], F32, kind="ExternalInput").ap()
    bv = nc.dram_tensor("bv", [F, 1], F32, kind="ExternalInput").ap()
    ow = nc.dram_tensor("ow", [E, E], IN_DT, kind="ExternalInput").ap()
    ob = nc.dram_tensor("ob", [1, E], F32, kind="ExternalInput").ap()
    # rows = [b, slot s] -> batch-b tokens [(core + 8*s)*GR, +GR)
    out = nc.dram_tensor("out", [B * TPB, E], F32, kind="ExternalOutput").ap()

    Exp = mybir.ActivationFunctionType.Exp

    with tile.TileContext(nc) as tc:
        with tc.tile_pool(name="persist", bufs=1) as persist, \
             tc.tile_pool(name="pAw", bufs=1) as pAw, \
             tc.tile_pool(name="pA", bufs=3) as pA, \
             tc.tile_pool(name="pBC", bufs=2) as pBC, \
             tc.tile_pool(name="pNr", bufs=2) as pNr, \
             tc.tile_pool(name="pD", bufs=1) as pD, \
             tc.tile_pool(name="pDo", bufs=2) as pDo, \
             tc.tile_pool(name="psA", bufs=1, space="PSUM") as psA, \
             tc.tile_pool(name="psT", bufs=1, space="PSUM") as psT, \
             tc.tile_pool(name="psS", bufs=2, space="PSUM") as psS, \
             tc.tile_pool(name="psAV", bufs=2, space="PSUM") as psAV, \
             tc.tile_pool(name="dramp", bufs=1, space="DRAM") as dramp:
            ident = persist.tile([P, P], CDT)
            make_identity(nc, ident)
            bq_sb = persist.tile([P, 1], F32)
            bk_sb = persist.tile([P, 1], F32)
            bv_sb = persist.tile([P, 1], F32)
            nc.sync.dma_start(bq_sb, bq)
            nc.sync.dma_start(bk_sb, bk)
            nc.sync.dma_start(bv_sb, bv)
            ob_row = persist.tile([1, E], F32)
            nc.sync.dma_start(ob_row, ob)
            obb = persist.tile([P, E], F32)
            nc.gpsimd.partition_broadcast(obb, ob_row)

            qfm = persist.tile([P, T], CDT)     # q^T (both heads stacked)
            kfm = persist.tile([P, T], CDT)     # k^T (rows 0:64 A, 64:128 B)
            # v token-major per 128-token chunk; per head 128 stationary
            # cols: [v (64) | ones | tail], head A tail = head B v rows,
            # head B tail = zeros. Head A lhsT = cols 0:128, B = 65:193.
            vtm = persist.tile([P, T // P, 194], CDT)
            ones1 = persist.tile([P, 1], F32)
            nc.vector.memset(ones1, 1.0)
            nc.vector.tensor_copy(vtm[:, :, 64], ones1.to_broadcast([P, T // P]))
            nc.vector.tensor_copy(vtm[:, :, 129], ones1.to_broadcast([P, T // P]))
            nc.vector.memset(vtm[:, :, 130:194], 0.0)
            attnA = persist.tile([64, T], CDT)  # head A attn out^T (normalized)
            attnB = persist.tile([64, T], CDT)

            wq_sb = pAw.tile([P, KO, F], CDT)
            wk_sb = pAw.tile([P, KO, F], CDT)
            wv_sb = pAw.tile([P, KO, F], CDT)
            nc.sync.dma_start(wq_sb, wq.rearrange("(ko p) f -> p ko f", p=P))
            xTr = xT.rearrange("(ko p) t -> p ko t", p=P)
            wkv_loaded = []

            # all-to-all staging: block r holds the slot-s granule
            # g = r + 8*s at offset s*GR; one collective per batch
            a2a_in = [dramp.tile([N_CORES, P, TPB], CDT, name=f"a2a_in{b}")
                      for b in range(B)]
            a2a_out = [dramp.tile([N_CORES, P, TPB], CDT, name=f"a2a_out{b}")
                       for b in range(B)]

            def phase_a(b):
                for tcx in range(NTC):
                    t0 = b * S + tcx * TC
                    xt = pA.tile([P, KO, TC], CDT, tag="xt")
                    if b == 0 and tcx == 0:
                        # split the first chunk's load per-ko so the very
                        # first matmul starts after ~128KB instead of ~1MB
                        for ko in range(KO):
                            nc.sync.dma_start(xt[:, ko],
                                              xTr[:, ko, t0:t0 + TC])
                    else:
                        nc.sync.dma_start(xt, xTr[:, :, t0:t0 + TC])
                    if not wkv_loaded:
                        nc.sync.dma_start(
                            wk_sb, wk.rearrange("(ko p) f -> p ko f", p=P))
                        nc.sync.dma_start(
                            wv_sb, wv.rearrange("(ko p) f -> p ko f", p=P))
                        wkv_loaded.append(True)
                    ps = psA.tile([P, TC], F32, tag="ps")
                    for ko in range(KO):
                        nc.tensor.matmul(ps, lhsT=wq_sb[:, ko], rhs=xt[:, ko],
                                         start=(ko == 0), stop=(ko == KO - 1))
                    nc.vector.tensor_scalar_add(qfm[:, t0:t0 + TC], ps, bq_sb)
                    ps = psT.tile([P, TC], F32, tag="tr")
                    for ko in range(KO):
                        nc.tensor.matmul(ps, lhsT=wk_sb[:, ko], rhs=xt[:, ko],
                                         start=(ko == 0), stop=(ko == KO - 1))
                    nc.vector.tensor_scalar_add(kfm[:, t0:t0 + TC], ps, bk_sb)
                    ps = psA.tile([P, TC], F32, tag="ps")
                    for ko in range(KO):
                        nc.tensor.matmul(ps, lhsT=wv_sb[:, ko], rhs=xt[:, ko],
                                         start=(ko == 0), stop=(ko == KO - 1))
                    vfm = pA.tile([P, TC], CDT, tag="vfm")
                    nc.vector.tensor_scalar_add(vfm, ps, bv_sb)
                    for sub in range(TC // P):
                        pst = psT.tile([P, P], CDT, tag="tr")
                        nc.tensor.transpose(pst, vfm[:, sub * P:(sub + 1) * P],
                                            ident)
                        c = (t0 + sub * P) // P
                        nc.vector.tensor_copy(vtm[:, c, 0:64], pst[:, 0:64])
                        nc.vector.tensor_copy(vtm[:, c, 65:129], pst[:, 64:128])

            def phase_bc(b, q_lo, q_hi):
                for qi in range(q_lo, q_hi):
                    q0 = b * S + qi * Q2
                    eA = pBC.tile([P, KC, Q2], CDT, tag="expA")
                    eB = pBC.tile([P, KC, Q2], CDT, tag="expB")
                    for kg in range(KC // G4):
                        sA = psS.tile([P, G4, Q2], F32, tag="sS")
                        sB = psS.tile([P, G4, Q2], F32, tag="sS")
                        for j in range(G4):
                            kc = kg * G4 + j
                            k0 = b * S + kc * P
                            # both heads: K=64 row tiles at partitions 0/64
                            nc.tensor.matmul(
                                sA[:, j], lhsT=kfm[0:64, k0:k0 + P],
                                rhs=qfm[0:64, q0:q0 + Q2],
                                start=True, stop=True)
                            nc.tensor.matmul(
                                sB[:, j], lhsT=kfm[64:128, k0:k0 + P],
                                rhs=qfm[64:128, q0:q0 + Q2],
                                start=True, stop=True)
                        g0 = kg * G4
                        nc.scalar.activation(eA[:, g0:g0 + G4], sA, Exp,
                                             scale=SCALE)
                        nc.scalar.activation(eB[:, g0:g0 + G4], sB, Exp,
                                             scale=SCALE)
                    pvA = psAV.tile([P, Q2], F32, tag="av")
                    pvB = psAV.tile([P, Q2], F32, tag="av")
                    for kc in range(KC):
                        c = (b * S) // P + kc
                        nc.tensor.matmul(pvA, lhsT=vtm[:, c, 0:128],
                                         rhs=eA[:, kc],
                                         start=(kc == 0), stop=(kc == KC - 1))
                        nc.tensor.matmul(pvB, lhsT=vtm[:, c, 65:193],
                                         rhs=eB[:, kc],
                                         start=(kc == 0), stop=(kc == KC - 1))
                    # row 64 = softmax denominators (raw); 65:128 garbage/0
                    dsb = pNr.tile([P, 2, Q2], F32, tag="dsb")
                    nc.vector.tensor_copy(dsb[64:65, 0], pvA[64:65])
                    nc.vector.tensor_copy(dsb[64:65, 1], pvB[64:65])
                    den0 = pNr.tile([1, 2, Q2], F32, tag="den0")
                    nc.sync.dma_start(den0, dsb[64:65])   # partition 64 -> 0
                    den = pNr.tile([64, 2, Q2], F32, tag="den")
                    nc.gpsimd.partition_broadcast(den, den0)
                    nc.vector.reciprocal_approx_fast(den, den)
                    nc.vector.tensor_mul(attnA[:, q0:q0 + Q2], pvA[0:64],
                                         den[:, 0])
                    nc.vector.tensor_mul(attnB[:, q0:q0 + Q2], pvB[0:64],
                                         den[:, 1])
                    for j in range(Q2 // GR):
                        g = qi * (Q2 // GR) + j    # batch-local granule
                        r, sl = g % N_CORES, g // N_CORES
                        gq = b * S + g * GR
                        nc.sync.dma_start(
                            a2a_in[b][r, 0:64, sl * GR:(sl + 1) * GR],
                            attnA[:, gq:gq + GR])
                        nc.sync.dma_start(
                            a2a_in[b][r, 64:128, sl * GR:(sl + 1) * GR],
                            attnB[:, gq:gq + GR])

            def send_a2a(b):
                nc.gpsimd.collective_compute(
                    "AllToAll", mybir.AluOpType.bypass,
                    replica_groups=[list(range(N_CORES))],
                    ins=[a2a_in[b].opt()], outs=[a2a_out[b].opt()])

            ow_sb = pD.tile([P, KO, E], CDT)

            def phase_d(b, sl):
                ga = pD.tile([P, N_CORES, GR], CDT, name=f"ga{b}{sl}")
                a2a_out_r = a2a_out[b].rearrange("c p t -> p c t")
                nc.sync.dma_start(ga, a2a_out_r[:, :, sl * GR:(sl + 1) * GR])
                for n2 in range(E // 512):
                    if n2 % 2 == 0:
                        pso = psA.tile([P, 512], F32, tag="ps")
                    else:
                        pso = psT.tile([P, 512], F32, tag="tr")
                    for r in range(N_CORES):
                        nc.tensor.matmul(
                            pso, lhsT=ga[:, r],
                            rhs=ow_sb[:, r, n2 * 512:(n2 + 1) * 512],
                            start=(r == 0), stop=(r == N_CORES - 1))
                    osb = pDo.tile([GR, 512], F32, tag="osb")
                    nc.vector.tensor_add(osb, pso[0:GR],
                                         obb[0:GR, n2 * 512:(n2 + 1) * 512])
                    r0 = b * TPB + sl * GR
                    nc.sync.dma_start(
                        out[r0:r0 + GR, n2 * 512:(n2 + 1) * 512], osb)

            phase_a(0)
            phase_bc(0, 0, NQ)
            send_a2a(0)
            nc.sync.dma_start(ow_sb, ow.rearrange("(r p) e -> p r e", p=P))
            if B > 1:
                phase_a(1)
                phase_bc(1, 0, NQ)
                send_a2a(1)
            for sl in range(SLOTS):
                phase_d(0, sl)
            if B > 1:
                for sl in range(SLOTS):
                    phase_d(1, sl)

    nc.compile()
    return nc


def make_in_maps(x, qkv_w, qkv_b, o_w, o_b, B=B_FULL, S=S_FULL,
                 compute=COMPUTE):
    """Host-side sharding: full inputs -> per-core input dicts."""
    T = B * S
    idt = ml_dtypes.bfloat16 if compute == "bf16" else np.float32
    x = np.asarray(x, dtype=np.float32)
    qkv_w = np.asarray(qkv_w, dtype=np.float32).astype(idt)
    qkv_b = np.asarray(qkv_b, dtype=np.float32)
    o_w = np.ascontiguousarray(np.asarray(o_w, dtype=np.float32).astype(idt))
    o_b = np.asarray(o_b, dtype=np.float32).reshape(1, E)
    xT = np.ascontiguousarray(x.reshape(T, E).T.astype(idt))
    in_maps = []
    for i in range(N_CORES):
        c0 = i * F
        in_maps.append({
            "xT": xT,
            "wq": np.ascontiguousarray(qkv_w[:, c0:c0 + F]),
            "wk": np.ascontiguousarray(qkv_w[:, E + c0:E + c0 + F]),
            "wv": np.ascontiguousarray(qkv_w[:, 2 * E + c0:2 * E + c0 + F]),
            "bq": np.ascontiguousarray(qkv_b[c0:c0 + F].reshape(F, 1)),
            "bk": np.ascontiguousarray(qkv_b[E + c0:E + c0 + F].reshape(F, 1)),
            "bv": np.ascontiguousarray(
                qkv_b[2 * E + c0:2 * E + c0 + F].reshape(F, 1)),
            "ow": o_w,
            "ob": o_b,
        })
    return in_maps


def gather_out(results, B=B_FULL, S=S_FULL):
    """Per-core [B*TPB, E] slices -> full [B, S, E].

    Core c, batch b, slot s rows map to batch tokens
    [(c + 8*s)*GR, (c + 8*s + 1)*GR).
    """
    TPB = S // N_CORES
    SLOTS = TPB // GR
    full = np.empty((B, S, E), dtype=np.float32)
    for c in range(N_CORES):
        r = results[c]["out"]
        for b in range(B):
            for s in range(SLOTS):
                g = c + N_CORES * s
                full[b, g * GR:(g + 1) * GR] = \
                    r[b * TPB + s * GR:b * TPB + (s + 1) * GR]
    return full


_NC_CACHE = {}


def _get_nc(B=B_FULL, S=S_FULL):
    key = (B, S, COMPUTE)
    if key not in _NC_CACHE:
        _NC_CACHE[key] = build_nc(B, S, COMPUTE)
    return _NC_CACHE[key]


def kernel(x, qkv_w, qkv_b, o_w, o_b):
    B, S, _ = np.asarray(x).shape
    nc = _get_nc(B, S)
    in_maps = make_in_maps(x, qkv_w, qkv_b, o_w, o_b, B, S)
    res = bass_utils.run_bass_kernel_spmd(
        nc, in_maps, core_ids=list(range(N_CORES)))
    return gather_out(res.results, B, S)


# revision 20
# speedup vs baseline: 1.2558x; 1.0009x over previous
"""Multi-head attention (B=2, S=2048, E=1024, H=16) on 8 Trainium2 NeuronCores.

Sharding: tensor-parallel over heads — core i owns heads (2i, 2i+1).
  Phase A  (per core, per batch): q/k/v projections for its 2 heads,
            feature-major [128 = 2x64 head-features, tokens]; v is
            PE-transposed to token-major with a ones column appended per
            head (softmax-denominator trick) and zero padding to 128
            stationary columns per head (enables fast weight loads).
  Phase B/C (per core, per batch): scores^T for both heads via K=64 PE row
            tiles at row positions 0/64 (no zero-padding waste); exp on
            ScalarE straight out of PSUM (softmax without max-subtraction —
            scores are O(1) for these inputs); AV matmul with the ones
            column so the denominator falls out of the same fp32
            accumulation; normalize = one partition-64->0 DMA hop, gpsimd
            partition-broadcast of the raw denominators, full-lane DVE
            approx-reciprocal, fused multiply into the AllToAll staging
            buffer.
  AllToAll: bf16 collectives re-sharding head-parallel [128 feat, tokens]
            to token-parallel [all 1024 feat, tokens/8]. Tokens map to
            cores in 128-token granules, block = granule % 8, so batch 1's
            collective can split into two slot-halves: the first half
            overlaps the last attention q-tiles, the second overlaps the
            (deferred) batch-0 output projection.
  Phase D  (per core, per batch, per slot): output projection for one
            128-token granule; emitted last so it backfills PE idle time
            during the final collective.

Batches are emitted interleaved so the Tile scheduler fills ScalarE-bound
gaps in one batch's attention with the other batch's projection matmuls.
Matmuls run in bf16 (full-rate + FWL weight loads); inputs are cast to bf16
on the host; PSUM accumulation is fp32 throughout.
"""

import numpy as np
import ml_dtypes

import concourse.bass as bass
import concourse.mybir as mybir
import concourse.tile as tile
from concourse import bacc
from concourse import bass_utils
from concourse.masks import make_identity

F32 = mybir.dt.float32
BF16 = mybir.dt.bfloat16
F32R = mybir.dt.float32r
N_CORES = 8
P = 128

COMPUTE = "bf16"              # "bf16" (fast) or "f32r" (precise fallback)

# Full problem dims (hardcoded per the harness contract)
B_FULL, S_FULL, E, H, D = 2, 2048, 1024, 16, 64
HPC = H // N_CORES            # heads per core = 2
F = HPC * D                   # feature cols per core = 128
SCALE = D ** -0.5
GR = 128                      # token granule for output sharding


def build_nc(B=B_FULL, S=S_FULL, compute=COMPUTE):
    CDT = BF16 if compute == "bf16" else F32R
    IN_DT = BF16 if compute == "bf16" else F32
    T = B * S                 # tokens
    KO = E // P               # 8 contraction chunks over embed
    TC = min(512, S)          # phase-A token chunk
    NTC = S // TC             # chunks per batch
    Q2 = min(512, S)          # q tile
    NQ = S // Q2
    KC = S // P               # k chunks per batch
    G4 = max(1, 1024 // Q2)   # kc group per exp call (N=1024 per ACTIVATE)
    TPB = S // N_CORES        # tokens per core PER BATCH for output proj
    SLOTS = TPB // GR         # granule slots per core per batch (2)

    nc = bacc.Bacc("TRN2", target_bir_lowering=False, debug=False,
                   num_devices=N_CORES)

    xT = nc.dram_tensor("xT", [E, T], IN_DT, kind="ExternalInput").ap()
    wq = nc.dram_tensor("wq", [E, F], IN_DT, kind="ExternalInput").ap()
    wk = nc.dram_tensor("wk", [E, F], IN_DT, kind="ExternalInput").ap()
    wv = nc.dram_tensor("wv", [E, F], IN_DT, kind="ExternalInput").ap()
    bq = nc.dram_tensor("bq", [F, 1], F32, kind="ExternalInput").ap()
    bk = nc.dram_tensor("bk", [F, 1], F32, kind="ExternalInput").ap()
    bv = nc.dram_tensor("bv", [F, 1], F32, kind="ExternalInput").ap()
    ow = nc.dram_tensor("ow", [E, E], IN_DT, kind="ExternalInput").ap()
    ob = nc.dram_tensor("ob", [1, E], F32, kind="ExternalInput").ap()
    # rows = [b, slot s] -> batch-b tokens [(core + 8*s)*GR, +GR)
    out = nc.dram_tensor("out", [B * TPB, E], F32, kind="ExternalOutput").ap()

    Exp = mybir.ActivationFunctionType.Exp

    with tile.TileContext(nc) as tc:
        with tc.tile_pool(name="persist", bufs=1) as persist, \
             tc.tile_pool(name="pAw", bufs=1) as pAw, \
             tc.tile_pool(name="pA", bufs=3) as pA, \
             tc.tile_pool(name="pBC", bufs=2) as pBC, \
             tc.tile_pool(name="pNr", bufs=2) as pNr, \
             tc.tile_pool(name="pD", bufs=1) as pD, \
             tc.tile_pool(name="pDo", bufs=2) as pDo, \
             tc.tile_pool(name="psA", bufs=1, space="PSUM") as psA, \
             tc.tile_pool(name="psT", bufs=1, space="PSUM") as psT, \
             tc.tile_pool(name="psS", bufs=2, space="PSUM") as psS, \
             tc.tile_pool(name="psAV", bufs=2, space="PSUM") as psAV, \
             tc.tile_pool(name="dramp", bufs=1, space="DRAM") as dramp:
            ident = persist.tile([P, P], CDT)
            make_identity(nc, ident)
            bq_sb = persist.tile([P, 1], F32)
            bk_sb = persist.tile([P, 1], F32)
            bv_sb = persist.tile([P, 1], F32)
            nc.sync.dma_start(bq_sb, bq)
            nc.sync.dma_start(bk_sb, bk)
            nc.sync.dma_start(bv_sb, bv)
            ob_row = persist.tile([1, E], F32)
            nc.sync.dma_start(ob_row, ob)
            obb = persist.tile([P, E], F32)
            nc.gpsimd.partition_broadcast(obb, ob_row)

            qfm = persist.tile([P, T], CDT)     # q^T (both heads stacked)
            kfm = persist.tile([P, T], CDT)     # k^T (rows 0:64 A, 64:128 B)
            # v token-major per 128-token chunk; per head 128 stationary
            # cols: [v (64) | ones | tail], head A tail = head B v rows,
            # head B tail = zeros. Head A lhsT = cols 0:128, B = 65:193.
            vtm = persist.tile([P, T // P, 194], CDT)
            ones1 = persist.tile([P, 1], F32)
            nc.vector.memset(ones1, 1.0)
            nc.vector.tensor_copy(vtm[:, :, 64], ones1.to_broadcast([P, T // P]))
            nc.vector.tensor_copy(vtm[:, :, 129], ones1.to_broadcast([P, T // P]))
            nc.vector.memset(vtm[:, :, 130:194], 0.0)
            attnA = persist.tile([64, T], CDT)  # head A attn out^T (normalized)
            attnB = persist.tile([64, T], CDT)

            wq_sb = pAw.tile([P, KO, F], CDT)
            wk_sb = pAw.tile([P, KO, F], CDT)
            wv_sb = pAw.tile([P, KO, F], CDT)
            nc.sync.dma_start(wq_sb, wq.rearrange("(ko p) f -> p ko f", p=P))
            xTr = xT.rearrange("(ko p) t -> p ko t", p=P)
            wkv_loaded = []

            # all-to-all staging: block r holds the slot-s granule
            # g = r + 8*s at offset s*GR; one collective per batch
            a2a_in = [dramp.tile([N_CORES, P, TPB], CDT, name=f"a2a_in{b}")
                      for b in range(B)]
            a2a_out = [dramp.tile([N_CORES, P, TPB], CDT, name=f"a2a_out{b}")
                       for b in range(B)]

            def phase_a(b):
                for tcx in range(NTC):
                    t0 = b * S + tcx * TC
                    xt = pA.tile([P, KO, TC], CDT, tag="xt")
                    if b == 0 and tcx == 0:
                        # split the first chunk's load per-ko so the very
                        # first matmul starts after ~128KB instead of ~1MB
                        for ko in range(KO):
                            nc.sync.dma_start(xt[:, ko],
                                              xTr[:, ko, t0:t0 + TC])
                    else:
                        nc.sync.dma_start(xt, xTr[:, :, t0:t0 + TC])
                    if not wkv_loaded:
                        nc.sync.dma_start(
                            wk_sb, wk.rearrange("(ko p) f -> p ko f", p=P))
                        nc.sync.dma_start(
                            wv_sb, wv.rearrange("(ko p) f -> p ko f", p=P))
                        wkv_loaded.append(True)
                    ps = psA.tile([P, TC], F32, tag="ps")
                    for ko in range(KO):
                        nc.tensor.matmul(ps, lhsT=wq_sb[:, ko], rhs=xt[:, ko],
                                         start=(ko == 0), stop=(ko == KO - 1))
                    nc.vector.tensor_scalar_add(qfm[:, t0:t0 + TC], ps, bq_sb)
                    ps = psT.tile([P, TC], F32, tag="tr")
                    for ko in range(KO):
                        nc.tensor.matmul(ps, lhsT=wk_sb[:, ko], rhs=xt[:, ko],
                                         start=(ko == 0), stop=(ko == KO - 1))
                    nc.vector.tensor_scalar_add(kfm[:, t0:t0 + TC], ps, bk_sb)
                    ps = psA.tile([P, TC], F32, tag="ps")
                    for ko in range(KO):
                        nc.tensor.matmul(ps, lhsT=wv_sb[:, ko], rhs=xt[:, ko],
                                         start=(ko == 0), stop=(ko == KO - 1))
                    vfm = pA.tile([P, TC], CDT, tag="vfm")
                    nc.vector.tensor_scalar_add(vfm, ps, bv_sb)
                    for sub in range(TC // P):
                        pst = psT.tile([P, P], CDT, tag="tr")
                        nc.tensor.transpose(pst, vfm[:, sub * P:(sub + 1) * P],
                                            ident)
                        c = (t0 + sub * P) // P
                        nc.vector.tensor_copy(vtm[:, c, 0:64], pst[:, 0:64])
                        nc.vector.tensor_copy(vtm[:, c, 65:129], pst[:, 64:128])

            def phase_bc(b, q_lo, q_hi):
                for qi in range(q_lo, q_hi):
                    q0 = b * S + qi * Q2
                    eA = pBC.tile([P, KC, Q2], CDT, tag="expA")
                    eB = pBC.tile([P, KC, Q2], CDT, tag="expB")
                    for kg in range(KC // G4):
                        sA = psS.tile([P, G4, Q2], F32, tag="sS")
                        sB = psS.tile([P, G4, Q2], F32, tag="sS")
                        for j in range(G4):
                            kc = kg * G4 + j
                            k0 = b * S + kc * P
                            # both heads: K=64 row tiles at partitions 0/64
                            nc.tensor.matmul(
                                sA[:, j], lhsT=kfm[0:64, k0:k0 + P],
                                rhs=qfm[0:64, q0:q0 + Q2],
                                start=True, stop=True)
                            nc.tensor.matmul(
                                sB[:, j], lhsT=kfm[64:128, k0:k0 + P],
                                rhs=qfm[64:128, q0:q0 + Q2],
                                start=True, stop=True)
                        g0 = kg * G4
                        nc.scalar.activation(eA[:, g0:g0 + G4], sA, Exp,
                                             scale=SCALE)
                        nc.scalar.activation(eB[:, g0:g0 + G4], sB, Exp,
                                             scale=SCALE)
                    pvA = psAV.tile([P, Q2], F32, tag="av")
                    pvB = psAV.tile([P, Q2], F32, tag="av")
                    for kc in range(KC):
                        c = (b * S) // P + kc
                        nc.tensor.matmul(pvA, lhsT=vtm[:, c, 0:128],
                                         rhs=eA[:, kc],
                                         start=(kc == 0), stop=(kc == KC - 1))
                        nc.tensor.matmul(pvB, lhsT=vtm[:, c, 65:193],
                                         rhs=eB[:, kc],
                                         start=(kc == 0), stop=(kc == KC - 1))
                    # row 64 = softmax denominators (raw); 65:128 garbage/0
                    dsb = pNr.tile([P, 2, Q2], F32, tag="dsb")
                    nc.vector.tensor_copy(dsb[64:65, 0], pvA[64:65])
                    nc.vector.tensor_copy(dsb[64:65, 1], pvB[64:65])
                    den0 = pNr.tile([1, 2, Q2], F32, tag="den0")
                    nc.sync.dma_start(den0, dsb[64:65])   # partition 64 -> 0
                    den = pNr.tile([64, 2, Q2], F32, tag="den")
                    nc.gpsimd.partition_broadcast(den, den0)
                    nc.vector.reciprocal_approx_fast(den, den)
                    nc.vector.tensor_mul(attnA[:, q0:q0 + Q2], pvA[0:64],
                                         den[:, 0])
                    nc.vector.tensor_mul(attnB[:, q0:q0 + Q2], pvB[0:64],
                                         den[:, 1])
                    for j in range(Q2 // GR):
                        g = qi * (Q2 // GR) + j    # batch-local granule
                        r, sl = g % N_CORES, g // N_CORES
                        gq = b * S + g * GR
                        nc.sync.dma_start(
                            a2a_in[b][r, 0:64, sl * GR:(sl + 1) * GR],
                            attnA[:, gq:gq + GR])
                        nc.sync.dma_start(
                            a2a_in[b][r, 64:128, sl * GR:(sl + 1) * GR],
                            attnB[:, gq:gq + GR])

            def send_a2a(b):
                nc.gpsimd.collective_compute(
                    "AllToAll", mybir.AluOpType.bypass,
                    replica_groups=[list(range(N_CORES))],
                    ins=[a2a_in[b].opt()], outs=[a2a_out[b].opt()])

            ow_sb = pD.tile([P, KO, E], CDT)

            def phase_d(b, sl):
                ga = pD.tile([P, N_CORES, GR], CDT, name=f"ga{b}{sl}")
                a2a_out_r = a2a_out[b].rearrange("c p t -> p c t")
                nc.sync.dma_start(ga, a2a_out_r[:, :, sl * GR:(sl + 1) * GR])
                for n2 in range(E // 512):
                    if n2 % 2 == 0:
                        pso = psA.tile([P, 512], F32, tag="ps")
                    else:
                        pso = psT.tile([P, 512], F32, tag="tr")
                    for r in range(N_CORES):
                        nc.tensor.matmul(
                            pso, lhsT=ga[:, r],
                            rhs=ow_sb[:, r, n2 * 512:(n2 + 1) * 512],
                            start=(r == 0), stop=(r == N_CORES - 1))
                    osb = pDo.tile([GR, 512], F32, tag="osb")
                    nc.vector.tensor_add(osb, pso[0:GR],
                                         obb[0:GR, n2 * 512:(n2 + 1) * 512])
                    r0 = b * TPB + sl * GR
                    nc.sync.dma_start(
                        out[r0:r0 + GR, n2 * 512:(n2 + 1) * 512], osb)

            phase_a(0)
            phase_bc(0, 0, NQ)
            send_a2a(0)
            nc.sync.dma_start(ow_sb, ow.rearrange("(r p) e -> p r e", p=P))
            if B > 1:
                phase_a(1)
                phase_bc(1, 0, NQ)
            # emitted before the second collective so batch-0's output
            # projection depends only on the first one
            for sl in range(SLOTS):
                phase_d(0, sl)
            if B > 1:
                send_a2a(1)
                for sl in range(SLOTS):
                    phase_d(1, sl)

    nc.compile()
    return nc


def make_in_maps(x, qkv_w, qkv_b, o_w, o_b, B=B_FULL, S=S_FULL,
                 compute=COMPUTE):
    """Host-side sharding: full inputs -> per-core input dicts."""
    T = B * S
    idt = ml_dtypes.bfloat16 if compute == "bf16" else np.float32
    x = np.asarray(x, dtype=np.float32)
    qkv_w = np.asarray(qkv_w, dtype=np.float32).astype(idt)
    qkv_b = np.asarray(qkv_b, dtype=np.float32)
    o_w = np.ascontiguousarray(np.asarray(o_w, dtype=np.float32).astype(idt))
    o_b = np.asarray(o_b, dtype=np.float32).reshape(1, E)
    xT = np.ascontiguousarray(x.reshape(T, E).T.astype(idt))
    in_maps = []
    for i in range(N_CORES):
        c0 = i * F
        in_maps.append({
            "xT": xT,
            "wq": np.ascontiguousarray(qkv_w[:, c0:c0 + F]),
            "wk": np.ascontiguousarray(qkv_w[:, E + c0:E + c0 + F]),
            "wv": np.ascontiguousarray(qkv_w[:, 2 * E + c0:2 * E + c0 + F]),
            "bq": np.ascontiguousarray(qkv_b[c0:c0 + F].reshape(F, 1)),
            "bk": np.ascontiguousarray(qkv_b[E + c0:E + c0 + F].reshape(F, 1)),
            "bv": np.ascontiguousarray(
                qkv_b[2 * E + c0:2 * E + c0 + F].reshape(F, 1)),
            "ow": o_w,
            "ob": o_b,
        })
    return in_maps


def gather_out(results, B=B_FULL, S=S_FULL):
    """Per-core [B*TPB, E] slices -> full [B, S, E].

    Core c, batch b, slot s rows map to batch tokens
    [(c + 8*s)*GR, (c + 8*s + 1)*GR).
    """
    TPB = S // N_CORES
    SLOTS = TPB // GR
    full = np.empty((B, S, E), dtype=np.float32)
    for c in range(N_CORES):
        r = results[c]["out"]
        for b in range(B):
            for s in range(SLOTS):
                g = c + N_CORES * s
                full[b, g * GR:(g + 1) * GR] = \
                    r[b * TPB + s * GR:b * TPB + (s + 1) * GR]
    return full


_NC_CACHE = {}


def _get_nc(B=B_FULL, S=S_FULL):
    key = (B, S, COMPUTE)
    if key not in _NC_CACHE:
        _NC_CACHE[key] = build_nc(B, S, COMPUTE)
    return _NC_CACHE[key]


def kernel(x, qkv_w, qkv_b, o_w, o_b):
    B, S, _ = np.asarray(x).shape
    nc = _get_nc(B, S)
    in_maps = make_in_maps(x, qkv_w, qkv_b, o_w, o_b, B, S)
    res = bass_utils.run_bass_kernel_spmd(
        nc, in_maps, core_ids=list(range(N_CORES)))
    return gather_out(res.results, B, S)
